# revision 1
# baseline (speedup 1.0000x reference)
"""Trainium2 Bass kernel for nn_ALNet (adaptive linear network forward).

Math: vals = x @ W + b  ([65536,256] @ [256,128] + [128]), then a 7-level
alternating min/max pairwise tree over the 128 leaf columns -> [B, 1].

Strategy (8 NeuronCores, data-parallel over the batch; per-core shard 8192):
  - Host prep: transpose each core's shard to xT [256, 8192] and cast to fp16
    so the contraction dim lands on SBUF partitions with zero on-device
    transposes (fp16 keeps elementwise value error ~2^-11, which the min/max
    tree preserves; PSUM accumulation is f32). Bit-reverse-permute W's
    columns (and b) so the alternating tree becomes 7 contiguous
    half-vs-half tensor_tensor ops on the free dim.
  - Device per core, tapered x super-loads [4096, 2048, 1024, 1024] rows
    (big loads for DMA bandwidth, small last loads for a short tail):
      PE:  per 1024-row PSUM group, bias is seeded by rank-1 ones^T @ bias_row
           matmuls (start=True), then x @ W accumulates as fp16 LDW+MM pairs;
           all K-half-0 MMs are emitted before K-half-1 MMs so the PE never
           stalls on the second K-half's DMA.
      ACT: evicts PSUM f32 -> SBUF bf16 (copy), one op per PSUM group,
           freeing PSUM banks at fine granularity.
      DVE: 7 halving min/max levels in bf16, batched over 2048-row blocks
           to amortize the ~151-cycle per-op overhead; final level writes
           f32 into the output staging tile.
  - Output staged as [128, 64] f32 (out[p, c] = batch row 128*c+p),
    de-interleaved on the host.
"""

import numpy as np

try:
    import concourse.bass as bass
except ImportError:  # pragma: no cover
    import sys

    sys.path.insert(0, "/opt/trn_rl_repo")
    import concourse.bass as bass

import concourse.mybir as mybir
import concourse.tile as tile
from concourse import bacc
from concourse.bass_utils import run_bass_kernel_spmd

F32 = mybir.dt.float32
BF16 = mybir.dt.bfloat16
F16 = mybir.dt.float16

B, F, NL = 65536, 256, 128
NCORES = 8
BS = B // NCORES  # 8192 batch rows per core

# Tree ops, deepest level first (reference folds reversed root->leaf list;
# the list [min,max,min,...] of length 7 is a palindrome).
_TREE_OPS = [
    mybir.AluOpType.min if i % 2 == 0 else mybir.AluOpType.max for i in range(7)
]


def _bitrev7_perm() -> np.ndarray:
    perm = np.zeros(NL, dtype=np.int64)
    for p in range(NL):
        r = 0
        for k in range(7):
            r |= ((p >> k) & 1) << (6 - k)
        perm[p] = r
    return perm


def build_nc(bs: int = BS, chunk: int = 1024):
    """bs = batch rows per core; chunk = rows per PSUM group."""
    assert chunk % 128 == 0 and bs % chunk == 0
    ncols = bs // 128

    nc = bacc.Bacc(None)
    xT = nc.declare_dram_parameter("xT", [F, bs], F16, isOutput=False)
    Wp = nc.declare_dram_parameter("Wp", [F, NL], F16, isOutput=False)
    brow = nc.declare_dram_parameter("brow", [1, 512], F16, isOutput=False)
    ones = nc.declare_dram_parameter("ones", [1, 128], F16, isOutput=False)
    out = nc.declare_dram_parameter("out", [128, ncols], F32, isOutput=True)

    with tile.TileContext(nc, pool_alloc_mode="queue") as tc:
        with (
            tc.tile_pool(name="const", bufs=1) as cpool,
            tc.tile_pool(name="xin", bufs=4) as xpool,
            tc.tile_pool(name="psum", bufs=max(2, (8 * 512) // chunk), space=bass.MemorySpace.PSUM) as ppool,
            tc.tile_pool(name="vals", bufs=2) as vpool,
            tc.tile_pool(name="lvl", bufs=2) as lpool,
            tc.tile_pool(name="ostage", bufs=1) as opool,
        ):
            # constants ride the scalar HWDGE ring so the x loads (sync
            # ring) start immediately
            brt = cpool.tile([1, 512], F16, tag="brt")
            ont = cpool.tile([1, 128], F16, tag="ont")
            w0t = cpool.tile([128, NL], F16, tag="w0t")
            w1t = cpool.tile([128, NL], F16, tag="w1t")
            nc.scalar.dma_start(out=brt[:], in_=brow[:])
            nc.scalar.dma_start(out=ont[:], in_=ones[:])
            nc.scalar.dma_start(out=w0t[:], in_=Wp[0:128, :])
            nc.scalar.dma_start(out=w1t[:], in_=Wp[128:256, :])


            ost = opool.tile([128, ncols], F32, tag="ost")


            # tapered super-loads: big first (bandwidth), small last (short tail)
            sups = []
            rem = bs
            plan = [4096, 2048, 1024, 1024]
            for p in plan:
                if rem >= p and p >= chunk:
                    sups.append(p)
                    rem -= p
            while rem:
                p = min(rem, sups[-1] if sups else bs)
                sups.append(p)
                rem -= p
            s0 = 0
            ocol = 0
            for s, sup_s in enumerate(sups):
                gps = sup_s // chunk
                tpb = chunk // 128
                x0 = xpool.tile([128, sup_s], F16, tag="x0", name=f"x0_{s}")
                x1 = xpool.tile([128, sup_s], F16, tag="x1", name=f"x1_{s}")
                nc.sync.dma_start(out=x0[:], in_=xT[0:128, s0 : s0 + sup_s])
                # split the K-half-1 load in two: subtile deps let the first
                # half's accumulating matmuls start while the second half is
                # still in flight
                if sup_s >= 2048:
                    h = sup_s // 2
                    nc.sync.dma_start(out=x1[:, 0:h], in_=xT[128:256, s0 : s0 + h])
                    nc.sync.dma_start(
                        out=x1[:, h:sup_s], in_=xT[128:256, s0 + h : s0 + sup_s]
                    )
                else:
                    nc.sync.dma_start(out=x1[:], in_=xT[128:256, s0 : s0 + sup_s])
                s0 += sup_s

                pss = [
                    ppool.tile([128, chunk], F32, tag="ps", name=f"ps_{s}_{g}")
                    for g in range(gps)
                ]
                for g in range(gps):
                    for bank in range(chunk // 512):
                        nc.tensor.matmul(
                            pss[g][:, bass.ts(bank, 512)],
                            ont[:],
                            brt[:],
                            start=True,
                            stop=False,
                        )
                for g in range(gps):
                    for t in range(tpb):
                        xsl = bass.ds(g * chunk + t * 128, 128)
                        nc.tensor.matmul(
                            pss[g][:, bass.ts(t, 128)],
                            x0[:, xsl],
                            w0t[:],
                            start=False,
                            stop=False,
                        )
                for g in range(gps):
                    for t in range(tpb):
                        xsl = bass.ds(g * chunk + t * 128, 128)
                        nc.tensor.matmul(
                            pss[g][:, bass.ts(t, 128)],
                            x1[:, xsl],
                            w1t[:],
                            start=False,
                            stop=(t % 4 == 3),
                        )

                tb = min(max(1, 2048 // chunk), gps)
                assert gps % tb == 0
                for gp in range(gps // tb):
                    v = vpool.tile([128, tb * chunk], BF16, tag="v", name=f"v_{s}_{gp}")
                    # evict at 512-col (PSUM bank) granularity: each bank's
                    # eviction fires as soon as its stop-matmul lands
                    # (subtile deps), recycling PSUM banks sooner and letting
                    # the final tree start earlier after the last matmul
                    ew = 512
                    for q in range(tb):
                        for e0 in range(0, chunk, ew):
                            nc.scalar.copy(
                                out=v[:, q * chunk + e0 : q * chunk + e0 + ew],
                                in_=pss[tb * gp + q][:, e0 : e0 + ew],
                            )
                    nblk = tb * tpb
                    cur = v
                    w = NL // 2
                    for lvl, op in enumerate(_TREE_OPS):
                        r = cur[:].rearrange(
                            "p (blk two h) -> p blk two h", two=2, h=w
                        )
                        in0 = r[:, :, 0, :]
                        in1 = r[:, :, 1, :]
                        if lvl < 6:
                            nxt = lpool.tile(
                                [128, nblk * w], BF16, tag=f"lvl{lvl}", name=f"l{lvl}_{s}_{gp}"
                            )
                            outap = nxt[:].rearrange("p (blk h) -> p blk h", h=w)
                        else:
                            nxt = None
                            outap = ost[:, ocol : ocol + nblk].rearrange(
                                "p (blk h) -> p blk h", h=1
                            )
                            ocol += nblk
                        nc.vector.tensor_tensor(out=outap, in0=in0, in1=in1, op=op)
                        cur = nxt
                        w //= 2

            nc.sync.dma_start(out=out[:], in_=ost[:])

    nc.compile()
    return nc


_NC_CACHE: dict = {}


def _get_nc(bs=BS, chunk=1024):
    key = (bs, chunk)
    if key not in _NC_CACHE:
        _NC_CACHE[key] = build_nc(bs, chunk)
    return _NC_CACHE[key]


def prep_inputs(x: np.ndarray, W: np.ndarray, b: np.ndarray) -> list[dict]:
    perm = _bitrev7_perm()
    Wp = np.ascontiguousarray(W[:, perm]).astype(np.float16)
    bp = np.ascontiguousarray(b[perm]).astype(np.float16)
    brow = np.ascontiguousarray(np.tile(bp[None, :], (1, 4)))  # [1, 512]
    ones = np.ones((1, 128), dtype=np.float16)
    x = np.asarray(x, dtype=np.float32)
    in_maps = []
    for i in range(NCORES):
        xTi = np.ascontiguousarray(x[i * BS : (i + 1) * BS, :].T).astype(np.float16)
        in_maps.append({"xT": xTi, "Wp": Wp, "brow": brow, "ones": ones})
    return in_maps


def gather_outputs(results: list[dict]) -> np.ndarray:
    shards = []
    for i in range(NCORES):
        o = np.asarray(results[i]["out"])  # [128, BS//128]; o[p, c] = row 128c+p
        shards.append(o.T.reshape(BS))
    return np.concatenate(shards).reshape(B, 1).astype(np.float32)


def _setup_tracing():
    """Install the antenv.axon_hooks NTFF-profile shim (missing from this
    image) and neuter the artifact upload so traced runs stay local."""
    import sys as _sys
    import types

    import concourse.bass_utils as bu

    bu.upload_artifacts = lambda tmpdir: tmpdir
    try:
        from antenv.axon_hooks import get_axon_ntff_profile_hook  # noqa: F401

        return
    except ImportError:
        pass
    import antenv

    m = types.ModuleType("antenv.axon_hooks")
    _state = {"hook": None}
    m.set_axon_ntff_profile_hook = lambda h: _state.__setitem__("hook", h)
    m.get_axon_ntff_profile_hook = lambda: _state["hook"]
    _sys.modules["antenv.axon_hooks"] = m
    antenv.axon_hooks = m
    try:
        from trn_agent_boot.trn_boot import _ntff_profile_via_ctypes

        hook = _ntff_profile_via_ctypes("/opt/axon/libaxon_pjrt.so")
        if hook is not None:
            m.set_axon_ntff_profile_hook(hook)
    except Exception as e:  # pragma: no cover
        print("ntff hook install failed:", e)


def run_on_hw(x, W, b, trace: bool = False, **kwargs):
    if trace:
        _setup_tracing()
    nc = _get_nc()
    in_maps = prep_inputs(np.asarray(x), np.asarray(W), np.asarray(b))
    return run_bass_kernel_spmd(
        nc, in_maps, core_ids=list(range(NCORES)), trace=trace, **kwargs
    )


def kernel(x: np.ndarray, W: np.ndarray, b: np.ndarray) -> np.ndarray:
    res = run_on_hw(x, W, b, trace=False)
    return gather_outputs(res.results)

